# revision 24
# baseline (speedup 1.0000x reference)
"""Trainium2 Bass kernel for nn_FeatureContraction.

Computes out[b,c,w,x,v] = sum_i x[b,c,w,x,v,i] * node_attributes[b,c,i]
with B=C=128, X=3, Y=16 (wxv = 3*16*16 = 768, i = 16).

Strategy (8 NeuronCores, data-parallel over b; PE does the math):
  - x is uploaded as fp8 e3m4 (4 mantissa bits; rel err ~1.4e-2 vs the
    2e-2 gate), host-packed so each b-slice is one [128, 12288] image:
    partition p = (c32, i4) with c32 = c%32 within a 32-channel group,
    i4 = i%4 within an i-chunk; free axis = (g, k, w). Loaded as four
    [128, 3072] per-group quarter tiles in consumption order.
  - per (g, k): one matmul with a block-diagonal stationary
    S[(c32,i4), c32'] = delta * na[32g+c32, 4k+i4] and moving rhs
    x[(c32,i4), w]; the 4 i-chunks (k) accumulate in PSUM; output
    strip = psum partitions [32g, 32g+32).
    Group 3 (strip base 96) is inexpressible as an AP base partition
    (rust IR allows only 0/32/64), so it uses a [128, 64] stationary
    [0 | diag] at base 64 and is emitted FIRST: its start=True zeroes
    rows 64-95, which group 2's own start=True then overwrites.
  - stationaries are built ON DEVICE by the (otherwise idle) DVE:
    S = mask * na_col, with mask in {mask32 [128,32], mask64 [128,64]}
    constants and na_col a per-partition scalar from a host-packed
    [128, 256] bf16 table (64 KB) -- so no 2.6 MB stationary upload.
  - whole contraction per b-slice = 32 matmuls into one [128, 768]
    f32 PSUM image; ACT copies it to bf16; HWDGE streams it out.
  - HBM per core: 24 MiB x (fp8) + 3 MiB out -> ~80 us at 358 GB/s;
    PE moving-data ~58 us busy. DMA-bound by design.
"""

import os
import sys

for _p in ("/opt/trn_rl_repo",):
    if _p not in sys.path:
        sys.path.append(_p)

import ml_dtypes
import numpy as np

import concourse.bass as bass
import concourse.mybir as mybir
import concourse.tile as tile
from concourse import bacc
from concourse.bass_utils import run_bass_kernel_spmd

# Problem dims (hardcoded per spec)
B, C, X, Y = 128, 128, 3, 16
WXV = X * Y * Y          # 768
I = Y                    # 16 (contraction axis)
N_CORES = 8
B_LOC = B // N_CORES     # 16 b-slices per core

NG = 4                   # channel groups of 32 (PSUM col-strip aligned)
CG = C // NG             # 32 channels per group
NK = 4                   # i-chunks of 4: K = CG*4 = 128 partitions
IK = I // NK             # 4
W_H0 = 512               # h0 span: exactly one 2 KB f32 PSUM bank
W_H1 = WXV - W_H0        # 256: the last-drained half, kept small for the tail
GQ = NK * WXV            # 3072: one group's x columns per b-slice
SKB = 2 * CG + 3 * CG    # 160 stationary cols per (b, k): [g3w|g0|g1|g2]

F32 = mybir.dt.float32
BF16 = mybir.dt.bfloat16
F8E3 = mybir.dt.float8e3

X_DT = os.environ.get("FC_X_DT", "f8e3")  # "f8e3" | "bf16" for A/B tests
X_MYBIR_DT = {"f8e3": F8E3, "bf16": BF16}[X_DT]
X_NP_DT = {"f8e3": ml_dtypes.float8_e3m4, "bf16": ml_dtypes.bfloat16}[X_DT]

GORDER = (3, 0, 1, 2)    # wide group 3 first (see module docstring)

_COMPILED = None


def _build():
    nc = bacc.Bacc("TRN2", target_bir_lowering=False, debug=False,
                   num_devices=N_CORES)

    x_d = nc.dram_tensor("x", [B_LOC, 128, NG * GQ], X_MYBIR_DT,
                         kind="ExternalInput")
    nacol_d = nc.dram_tensor("nacol", [128, B_LOC * NG * NK], F32,
                             kind="ExternalInput")
    mask_d = nc.dram_tensor("mask", [128, 3 * CG], BF16,
                            kind="ExternalInput")
    out_d = nc.dram_tensor("out", [B_LOC, C, WXV], BF16,
                           kind="ExternalOutput")

    with tile.TileContext(nc) as tc:
        with (
            tc.tile_pool(name="const", bufs=1) as constp,
            tc.tile_pool(name="xp", bufs=14) as xp,
            tc.tile_pool(name="sp", bufs=4) as sp,
            tc.tile_pool(name="outp", bufs=4) as outp,
            tc.tile_pool(name="psp", bufs=4, space="PSUM") as psp,
        ):
            nacol = constp.tile([128, B_LOC * NG * NK], F32)
            mask = constp.tile([128, 3 * CG], BF16)  # [mask64 | mask32]
            # consts lead the sync ring (0.45 us ahead of x): avoids
            # touching the SWDGE ring at all, testing whether its drain
            # contributes to the end-of-program fence
            nc.sync.dma_start(mask[:], mask_d[:])
            nc.sync.dma_start(nacol[:], nacol_d[:])

            for b in range(B_LOC):
                # DVE builds this slice's stationaries from na_col table
                st = sp.tile([128, NK * SKB], BF16, tag="s")
                for g in GORDER:  # g3's sections first: PE needs them first
                    for k in range(NK):
                        j = (b * NG + g) * NK + k
                        col = nacol[:, j : j + 1]
                        if g == 3:
                            nc.vector.tensor_scalar_mul(
                                st[:, k * SKB : k * SKB + 2 * CG],
                                mask[:, : 2 * CG], col)
                        else:
                            o = k * SKB + 2 * CG + g * CG
                            nc.vector.tensor_scalar_mul(
                                st[:, o : o + CG],
                                mask[:, 2 * CG : 3 * CG], col)

                xts = {}
                last_b = b == B_LOC - 1
                for g in GORDER:
                    if last_b and g == 2:
                        # final quarter arrives as four per-k chunks so PE
                        # accumulates while the tail of the stream lands
                        xck = []
                        for kk in range(NK):
                            xc = xp.tile([128, WXV], X_MYBIR_DT, tag="xk")
                            nc.sync.dma_start(
                                xc[:],
                                x_d[b, :, g * GQ + kk * WXV :
                                    g * GQ + (kk + 1) * WXV])
                            xck.append(xc)
                        xts[g] = xck
                        continue
                    xt = xp.tile([128, GQ], X_MYBIR_DT, tag="x")
                    nc.sync.dma_start(xt[:], x_d[b, :, g * GQ : (g + 1) * GQ])
                    xts[g] = xt

                ps0 = psp.tile([128, W_H0], F32, tag="ps0")
                ps1 = psp.tile([128, W_H1], F32, tag="ps1")
                ps = {0: ps0, 1: ps1}
                hspan = {0: (0, W_H0), 1: (W_H0, WXV)}
                for g in GORDER:
                    if last_b and g == 2:
                        # k-outer so each chunk is consumed on arrival
                        for k in range(NK):
                            s0 = k * SKB + 2 * CG + g * CG
                            for h in range(2):
                                w0, w1 = hspan[h]
                                nc.tensor.matmul(
                                    ps[h][CG * g : CG * (g + 1), :],
                                    st[:, s0 : s0 + CG],
                                    xts[g][k][:, w0:w1],
                                    start=(k == 0),
                                    stop=(k == NK - 1),
                                )
                        continue
                    for h in range(2):
                        w0, w1 = hspan[h]
                        for k in range(NK):
                            if g == 3:
                                lhsT = st[:, k * SKB : k * SKB + 2 * CG]
                                oap = ps[h][2 * CG : 4 * CG, :]
                            else:
                                s0 = k * SKB + 2 * CG + g * CG
                                lhsT = st[:, s0 : s0 + CG]
                                oap = ps[h][CG * g : CG * (g + 1), :]
                            nc.tensor.matmul(
                                oap,
                                lhsT,
                                xts[g][:, k * WXV + w0 : k * WXV + w1],
                                start=(k == 0),
                                stop=(k == NK - 1),
                            )

                ot = outp.tile([C, WXV], BF16, tag="out")
                # drain the two PSUM halves on different engines so the
                # final slice's copies overlap (ACT h0 || DVE h1), and
                # store per half so the h0 half flies while h1 drains
                nc.scalar.copy(ot[:, 0:W_H0], ps[0][:])
                nc.vector.tensor_copy(ot[:, W_H0:WXV], ps[1][:])
                nc.scalar.dma_start(out_d[b, :, 0:W_H0], ot[:, 0:W_H0])
                # final slice: h1 half rides the (now idle) sync ring so
                # the two last stores drain in parallel; mid-stream it
                # must stay off the sync FIFO or it would stall x loads
                oeng = nc.sync if b == B_LOC - 1 else nc.scalar
                oeng.dma_start(out_d[b, :, W_H0:WXV], ot[:, W_H0:WXV])

    nc.compile()
    return nc


def _get_compiled():
    global _COMPILED
    if _COMPILED is None:
        _COMPILED = _build()
    return _COMPILED


def _make_in_maps(inputs: dict):
    x = np.asarray(inputs["x"], dtype=np.float32)
    na = np.asarray(inputs["node_attributes"], dtype=np.float32)

    # x[b, c, w, i] -> xq[b, p=(c32,i4), (g, k), w], cast first (cheaper
    # to transpose 1-2 B elems than 4 B)
    xq = x.reshape(B, C, WXV, I).astype(X_NP_DT)
    xq = xq.reshape(B, NG, CG, WXV, NK, IK)
    xq = np.ascontiguousarray(xq.transpose(0, 2, 5, 1, 4, 3))
    xq = xq.reshape(B, 128, NG * GQ)

    # na_col[p=(c32,i4), (b, g, k)] = na[b, 32g+c32, 4k+i4]
    nacol = na.reshape(B, NG, CG, NK, IK).transpose(2, 4, 0, 1, 3)
    nacol = np.ascontiguousarray(nacol).reshape(128, B * NG * NK)
    nacol = nacol.astype(np.float32)

    # masks: mask64[p, j] = (j >= 32) & (p//4 == j-32); mask32[p, m] = (p//4 == m)
    p4 = np.arange(128) // IK
    m32 = (p4[:, None] == np.arange(CG)[None, :])
    mask = np.concatenate(
        [np.zeros((128, CG), bool), m32, m32], axis=1
    ).astype(ml_dtypes.bfloat16)

    in_maps = []
    for kcore in range(N_CORES):
        b0 = kcore * B_LOC
        nci = nacol.reshape(128, B, NG * NK)[:, b0 : b0 + B_LOC]
        in_maps.append(
            {
                "x": xq[b0 : b0 + B_LOC],
                "nacol": np.ascontiguousarray(nci).reshape(128, -1),
                "mask": mask,
            }
        )
    return in_maps


def _gather(results) -> np.ndarray:
    out = np.concatenate([np.asarray(r["out"]) for r in results], axis=0)
    return out.astype(np.float32).reshape(B, C, X, Y, Y)


def _run(inputs: dict, trace: bool = False, trace_cores=None):
    in_maps = _make_in_maps(inputs)
    nc = _get_compiled()
    res = run_bass_kernel_spmd(
        nc,
        in_maps,
        core_ids=list(range(N_CORES)),
        trace=trace,
        trace_cores=trace_cores,
    )
    return _gather(res.results), res


def kernel(**inputs) -> np.ndarray:
    out, _ = _run(inputs, trace=False)
    return out


# revision 25
# speedup vs baseline: 1.0260x; 1.0260x over previous
"""Trainium2 Bass kernel for nn_FeatureContraction.

Computes out[b,c,w,x,v] = sum_i x[b,c,w,x,v,i] * node_attributes[b,c,i]
with B=C=128, X=3, Y=16 (wxv = 3*16*16 = 768, i = 16).

Strategy (8 NeuronCores, data-parallel over b; PE does the math):
  - x is uploaded as fp8 e3m4 (4 mantissa bits; rel err ~1.4e-2 vs the
    2e-2 gate), host-packed so each b-slice is one [128, 12288] image:
    partition p = (c32, i4) with c32 = c%32 within a 32-channel group,
    i4 = i%4 within an i-chunk; free axis = (g, k, w). Loaded as four
    [128, 3072] per-group quarter tiles in consumption order.
  - per (g, k): one matmul with a block-diagonal stationary
    S[(c32,i4), c32'] = delta * na[32g+c32, 4k+i4] and moving rhs
    x[(c32,i4), w]; the 4 i-chunks (k) accumulate in PSUM; output
    strip = psum partitions [32g, 32g+32).
    Group 3 (strip base 96) is inexpressible as an AP base partition
    (rust IR allows only 0/32/64), so it uses a [128, 64] stationary
    [0 | diag] at base 64 and is emitted FIRST: its start=True zeroes
    rows 64-95, which group 2's own start=True then overwrites.
  - stationaries are built ON DEVICE by the (otherwise idle) DVE:
    S = mask * na_col, with mask in {mask32 [128,32], mask64 [128,64]}
    constants and na_col a per-partition scalar from a host-packed
    [128, 256] bf16 table (64 KB) -- so no 2.6 MB stationary upload.
  - whole contraction per b-slice = 32 matmuls into one [128, 768]
    f32 PSUM image; ACT copies it to bf16; HWDGE streams it out.
  - HBM per core: 24 MiB x (fp8) + 3 MiB out -> ~80 us at 358 GB/s;
    PE moving-data ~58 us busy. DMA-bound by design.
"""

import os
import sys

for _p in ("/opt/trn_rl_repo",):
    if _p not in sys.path:
        sys.path.append(_p)

import ml_dtypes
import numpy as np

import concourse.bass as bass
import concourse.mybir as mybir
import concourse.tile as tile
from concourse import bacc
from concourse.bass_utils import run_bass_kernel_spmd

# Problem dims (hardcoded per spec)
B, C, X, Y = 128, 128, 3, 16
WXV = X * Y * Y          # 768
I = Y                    # 16 (contraction axis)
N_CORES = 8
B_LOC = B // N_CORES     # 16 b-slices per core

NG = 4                   # channel groups of 32 (PSUM col-strip aligned)
CG = C // NG             # 32 channels per group
NK = 4                   # i-chunks of 4: K = CG*4 = 128 partitions
IK = I // NK             # 4
W_H0 = 512               # h0 span: exactly one 2 KB f32 PSUM bank
W_H1 = WXV - W_H0        # 256: the last-drained half, kept small for the tail
GQ = NK * WXV            # 3072: one group's x columns per b-slice
SKB = 2 * CG + 3 * CG    # 160 stationary cols per (b, k): [g3w|g0|g1|g2]

F32 = mybir.dt.float32
BF16 = mybir.dt.bfloat16
F8E3 = mybir.dt.float8e3

X_DT = os.environ.get("FC_X_DT", "f8e3")  # "f8e3" | "bf16" for A/B tests
X_MYBIR_DT = {"f8e3": F8E3, "bf16": BF16}[X_DT]
X_NP_DT = {"f8e3": ml_dtypes.float8_e3m4, "bf16": ml_dtypes.bfloat16}[X_DT]

GORDER = (3, 0, 1, 2)    # wide group 3 first (see module docstring)

_COMPILED = None


def _build():
    nc = bacc.Bacc("TRN2", target_bir_lowering=False, debug=False,
                   num_devices=N_CORES)

    x_d = nc.dram_tensor("x", [B_LOC, 128, NG * GQ], X_MYBIR_DT,
                         kind="ExternalInput")
    nacol_d = nc.dram_tensor("nacol", [128, B_LOC * NG * NK], F32,
                             kind="ExternalInput")
    mask_d = nc.dram_tensor("mask", [128, 3 * CG], BF16,
                            kind="ExternalInput")
    out_d = nc.dram_tensor("out", [B_LOC, C, WXV], BF16,
                           kind="ExternalOutput")

    with tile.TileContext(nc) as tc:
        with (
            tc.tile_pool(name="const", bufs=1) as constp,
            tc.tile_pool(name="xp", bufs=14) as xp,
            tc.tile_pool(name="sp", bufs=4) as sp,
            tc.tile_pool(name="outp", bufs=4) as outp,
            tc.tile_pool(name="psp", bufs=4, space="PSUM") as psp,
        ):
            nacol = constp.tile([128, B_LOC * NG * NK], F32)
            mask = constp.tile([128, 3 * CG], BF16)  # [mask64 | mask32]
            # consts via SWDGE (gpsimd): the sync ring then carries ONLY
            # x quarters (stream starts at the body barrier), and the ACT
            # ring's table-load doesn't delay the S-build constants
            nc.gpsimd.dma_start(mask[:], mask_d[:])
            nc.gpsimd.dma_start(nacol[:], nacol_d[:])

            for b in range(B_LOC):
                # DVE builds this slice's stationaries from na_col table
                st = sp.tile([128, NK * SKB], BF16, tag="s")
                for g in GORDER:  # g3's sections first: PE needs them first
                    for k in range(NK):
                        j = (b * NG + g) * NK + k
                        col = nacol[:, j : j + 1]
                        if g == 3:
                            nc.vector.tensor_scalar_mul(
                                st[:, k * SKB : k * SKB + 2 * CG],
                                mask[:, : 2 * CG], col)
                        else:
                            o = k * SKB + 2 * CG + g * CG
                            nc.vector.tensor_scalar_mul(
                                st[:, o : o + CG],
                                mask[:, 2 * CG : 3 * CG], col)

                xts = {}
                last_b = b == B_LOC - 1
                for g in GORDER:
                    if last_b and g == 2:
                        # final quarter arrives as four per-k chunks so PE
                        # accumulates while the tail of the stream lands
                        xck = []
                        for kk in range(NK):
                            xc = xp.tile([128, WXV], X_MYBIR_DT, tag="xk")
                            nc.sync.dma_start(
                                xc[:],
                                x_d[b, :, g * GQ + kk * WXV :
                                    g * GQ + (kk + 1) * WXV])
                            xck.append(xc)
                        xts[g] = xck
                        continue
                    xt = xp.tile([128, GQ], X_MYBIR_DT, tag="x")
                    nc.sync.dma_start(xt[:], x_d[b, :, g * GQ : (g + 1) * GQ])
                    xts[g] = xt

                ps0 = psp.tile([128, W_H0], F32, tag="ps0")
                ps1 = psp.tile([128, W_H1], F32, tag="ps1")
                ps = {0: ps0, 1: ps1}
                hspan = {0: (0, W_H0), 1: (W_H0, WXV)}
                for g in GORDER:
                    if last_b and g == 2:
                        # k-outer so each chunk is consumed on arrival
                        for k in range(NK):
                            s0 = k * SKB + 2 * CG + g * CG
                            for h in range(2):
                                w0, w1 = hspan[h]
                                nc.tensor.matmul(
                                    ps[h][CG * g : CG * (g + 1), :],
                                    st[:, s0 : s0 + CG],
                                    xts[g][k][:, w0:w1],
                                    start=(k == 0),
                                    stop=(k == NK - 1),
                                )
                        continue
                    for h in range(2):
                        w0, w1 = hspan[h]
                        for k in range(NK):
                            if g == 3:
                                lhsT = st[:, k * SKB : k * SKB + 2 * CG]
                                oap = ps[h][2 * CG : 4 * CG, :]
                            else:
                                s0 = k * SKB + 2 * CG + g * CG
                                lhsT = st[:, s0 : s0 + CG]
                                oap = ps[h][CG * g : CG * (g + 1), :]
                            nc.tensor.matmul(
                                oap,
                                lhsT,
                                xts[g][:, k * WXV + w0 : k * WXV + w1],
                                start=(k == 0),
                                stop=(k == NK - 1),
                            )

                ot = outp.tile([C, WXV], BF16, tag="out")
                # drain the two PSUM halves on different engines so the
                # final slice's copies overlap (ACT h0 || DVE h1), and
                # store per half so the h0 half flies while h1 drains
                nc.scalar.copy(ot[:, 0:W_H0], ps[0][:])
                nc.vector.tensor_copy(ot[:, W_H0:WXV], ps[1][:])
                nc.scalar.dma_start(out_d[b, :, 0:W_H0], ot[:, 0:W_H0])
                # final slice: h1 half rides the (now idle) sync ring so
                # the two last stores drain in parallel; mid-stream it
                # must stay off the sync FIFO or it would stall x loads
                oeng = nc.sync if b == B_LOC - 1 else nc.scalar
                oeng.dma_start(out_d[b, :, W_H0:WXV], ot[:, W_H0:WXV])

    nc.compile()
    return nc


def _get_compiled():
    global _COMPILED
    if _COMPILED is None:
        _COMPILED = _build()
    return _COMPILED


def _make_in_maps(inputs: dict):
    x = np.asarray(inputs["x"], dtype=np.float32)
    na = np.asarray(inputs["node_attributes"], dtype=np.float32)

    # x[b, c, w, i] -> xq[b, p=(c32,i4), (g, k), w], cast first (cheaper
    # to transpose 1-2 B elems than 4 B)
    xq = x.reshape(B, C, WXV, I).astype(X_NP_DT)
    xq = xq.reshape(B, NG, CG, WXV, NK, IK)
    xq = np.ascontiguousarray(xq.transpose(0, 2, 5, 1, 4, 3))
    xq = xq.reshape(B, 128, NG * GQ)

    # na_col[p=(c32,i4), (b, g, k)] = na[b, 32g+c32, 4k+i4]
    nacol = na.reshape(B, NG, CG, NK, IK).transpose(2, 4, 0, 1, 3)
    nacol = np.ascontiguousarray(nacol).reshape(128, B * NG * NK)
    nacol = nacol.astype(np.float32)

    # masks: mask64[p, j] = (j >= 32) & (p//4 == j-32); mask32[p, m] = (p//4 == m)
    p4 = np.arange(128) // IK
    m32 = (p4[:, None] == np.arange(CG)[None, :])
    mask = np.concatenate(
        [np.zeros((128, CG), bool), m32, m32], axis=1
    ).astype(ml_dtypes.bfloat16)

    in_maps = []
    for kcore in range(N_CORES):
        b0 = kcore * B_LOC
        nci = nacol.reshape(128, B, NG * NK)[:, b0 : b0 + B_LOC]
        in_maps.append(
            {
                "x": xq[b0 : b0 + B_LOC],
                "nacol": np.ascontiguousarray(nci).reshape(128, -1),
                "mask": mask,
            }
        )
    return in_maps


def _gather(results) -> np.ndarray:
    out = np.concatenate([np.asarray(r["out"]) for r in results], axis=0)
    return out.astype(np.float32).reshape(B, C, X, Y, Y)


def _run(inputs: dict, trace: bool = False, trace_cores=None):
    in_maps = _make_in_maps(inputs)
    nc = _get_compiled()
    res = run_bass_kernel_spmd(
        nc,
        in_maps,
        core_ids=list(range(N_CORES)),
        trace=trace,
        trace_cores=trace_cores,
    )
    return _gather(res.results), res


def kernel(**inputs) -> np.ndarray:
    out, _ = _run(inputs, trace=False)
    return out
